# revision 1
# baseline (speedup 1.0000x reference)
"""BlockSparseLinear forward on 8 Trainium2 NeuronCores (bf16 pipeline).

Computes out = x @ (weight * expand(block_mask))^T + bias for
x [8192, 4096] f32, weight [4096, 4096] f32, bias [4096] f32,
block_mask [128, 128] int32 (32x32 blocks).

Sharding: data-parallel over rows of x -- each of the 8 cores gets a
1024-row slice of x and the full weight / bias / block_mask
(replicated).  No collectives; per-core out^T [4096, 1024] is
transposed and concatenated on the host.

Layout/precision strategy (vs the f32r baseline, 533.9us; this version
measures ~479us, ~444us of which is the PE floor of 2048 matmuls):
  * x and weight ship as bf16 (host-side dtype cast + pure index
    permutations).  bf16 matmuls run at the same 1 cycle/row as f32r,
    but the bf16 stationary enables the PE's Fast Weight Load path
    (f32r counts as FP32_HIGH, which disables FWL): LDWEIGHTS drops
    ~187ns -> ~97ns and hides under the matmul, taking the warm matmul
    cadence from 227ns to the 216ns floor.  DMA traffic halves.
  * Mask expansion is 2 single-DMA partition-broadcasts straight from
    the host-provided maskB layout (maskB[q,t,ob] = maskT[4t+q,ob], so
    every destination partition reads one contiguous run) -- no PE
    selection matmuls, nothing on the vector engine's critical path.
    PSUM is left entirely to the matmul accumulators.
  * 9 dependency-free warmup matmuls on a memset tile run during the
    framework preamble so the HAM clock-gate opens (1.2 -> 2.4 GHz)
    before the first real matmul.
  * Startup: the first 4 output tiles are interleaved across the 8
    x-chunks as they stream in (phase A), so the PE runs at full rate
    while x loads instead of stalling on the full contraction of
    output tile 0.  Remaining 28 output tiles run dense (phase B),
    with weight pieces prefetched two tiles ahead.
  * Per 128-output tile: DVE multiplies the weight tile by the
    partition-replicated mask (bf16, broadcast access pattern); 64
    bf16 matmuls [128x128]x[128x512] accumulate out^T in fp32 PSUM;
    bias is added during PSUM->SBUF eviction on the scalar engine.
    Phase-A evictions instead ride the vector engine so the Tile
    scheduler cannot queue them behind ring-gated DMA issue ops --
    their PSUM banks must free fast for the first phase-B tiles.
  * The last output tile's matmuls are ng-split so its first eviction
    and store overlap the remaining matmuls (shorter tail).

All reference arithmetic -- mask application, matmuls, bias add --
runs on device; host work is dtype casts and index permutations.
"""
import os
import sys

import ml_dtypes
import numpy as np

sys.path.insert(0, "/opt/trn_rl_repo")

from contextlib import ExitStack

import concourse.bass as bass
import concourse.mybir as mybir
import concourse.tile as tile
from concourse import bacc
from concourse.bass_utils import run_bass_kernel_spmd

N_CORES = 8
BS = 32
P = 128

# Filled by kernel() after a profiled run (test harness convenience).
LAST_EXEC_TIME_NS = None
LAST_RESULTS = None

F32 = mybir.dt.float32
BF16 = mybir.dt.bfloat16
I32 = mybir.dt.int32


def _build_program(n_rows, IN, OUT):
    """Per-core SPMD program.  Inputs:
      xq     [NQ, NG, 128, QI, NFREE] bf16  xq[c,ng,p,it,n] = x[ng*NFREE+n, (c*QI+it)*128+p]
      wq     [OT, 128, IT, 128] bf16        wq[ot,p,it,o]   = weight[ot*128+o, it*128+p]
      maskT  [IB, OB] bf16                  block_mask.T
      bias_r [128, OT] f32                  bias_r[p,ot]    = bias[ot*128+p]
    Output outT [OUT, n_rows] f32 (outT[o,n] = out[n,o])."""
    IT = IN // P           # contraction tiles
    OT = OUT // P          # output tiles
    TG = IT // 4           # tile groups (one masked-weight tile per tg)
    NFREE = min(512, n_rows)
    NG = n_rows // NFREE
    IB = IN // BS
    OB = OUT // BS
    QI = 4                 # i-tiles per x chunk
    NQ = IT // QI          # x chunks; phase A round c consumes chunk c
    AOT = 4                # output tiles interleaved in phase A
    PIECE = 4              # i-tiles per weight DMA (= one tile group)
    NPC = IT // PIECE      # weight pieces per output tile
    assert IB <= P and OB <= P and QI * NQ == IT and TG == NQ
    assert PIECE == 4 and NPC == TG

    nc = bacc.Bacc("TRN2", target_bir_lowering=False, debug=False,
                   num_devices=N_CORES)
    xq_d = nc.dram_tensor("xq", [NQ, NG, P, QI, NFREE], BF16,
                          kind="ExternalInput")
    wq_d = nc.dram_tensor("wq", [OT, P, IT, P], BF16, kind="ExternalInput")
    # maskB[q, t, ob] = maskT[4t + q, ob]: partition p = 32q + r of mrep
    # reads the contiguous 8KB row maskB[q] (fast broadcast descriptors).
    mask_d = nc.dram_tensor("maskB", [4, IT, OB], BF16, kind="ExternalInput")
    bias_d = nc.dram_tensor("bias_r", [P, OT], F32, kind="ExternalInput")
    out_d = nc.dram_tensor("outT", [OUT, n_rows], F32, kind="ExternalOutput")

    with tile.TileContext(nc) as tc, ExitStack() as ctx:
        const = ctx.enter_context(tc.tile_pool(name="const", bufs=1))
        xtp = ctx.enter_context(tc.tile_pool(name="xt", bufs=1))
        mrp = ctx.enter_context(tc.tile_pool(name="mrep", bufs=1))
        wnt = ctx.enter_context(tc.tile_pool(name="wnt", bufs=44))
        # 12 bufs: the ng-split passes (phase A round 0, first/last phase-B
        # tiles) keep up to 8 masked-weight tiles live at once, plus
        # run-ahead slack.
        wtm = ctx.enter_context(tc.tile_pool(name="wtm", bufs=12))
        osb = ctx.enter_context(tc.tile_pool(name="osb", bufs=3))
        ppo = ctx.enter_context(tc.tile_pool(name="ppo", bufs=8, space="PSUM"))

        # ---- PE warm-up: dependency-light matmuls during the framework
        # preamble so the HAM clock-gate opens (K=8/8 @ 2.4GHz) before the
        # first real matmul instead of ~3.4us into the main loop.
        dum = const.tile([P, P + NFREE], BF16)
        nc.vector.memset(dum[:], 0.0)
        wup = ppo.tile([P, NFREE], F32, tag="ppo", name="warmup")
        for _ in range(9):
            nc.tensor.matmul(wup[:], dum[:, 0:P], dum[:, P:P + NFREE],
                             start=True, stop=True)

        # ---- mask partition-broadcast, split across both HWDGE rings ----
        # mrep[p, it, ob] = maskT[4*it + p//32, ob]: partition p of i-tile
        # it holds input block ib = 4*it + p//32.  Each destination
        # partition reads one contiguous maskB row (fast descriptors); the
        # two t-halves are separate tiles so the first masked-weight
        # multiply only waits on the first half.
        HT = IT // 2
        mrepA = mrp.tile([P, HT, OB], BF16, name="mrepA")
        mrepB = mrp.tile([P, HT, OB], BF16, name="mrepB")

        def mask_bcast_src(t0, t1):
            # [4, HT, OB] slice -> [4, 32(bcast), HT, OB]: dst partition
            # p = 32q + r reads maskB[q, t0:t1] (contiguous), one DMA total.
            return mask_d[:, t0:t1] \
                .rearrange("q (t x) o -> q x t o", x=1) \
                .broadcast_to([4, 32, t1 - t0, OB])

        nc.scalar.dma_start(mrepA[:], mask_bcast_src(0, HT))

        def load_piece(ot, pc):
            t = wnt.tile([P, PIECE, P], BF16, tag="wnt", name=f"w_{ot}_{pc}")
            nc.scalar.dma_start(t[:], wq_d[ot, :, pc * PIECE:(pc + 1) * PIECE, :])
            return t

        # Phase A weights piece-major so round 0's dependencies land first.
        wpiece = {}
        for pc in range(NPC):
            for ot in range(AOT):
                wpiece[(ot, pc)] = load_piece(ot, pc)
            if pc == 1:
                # second mask half: needed from round TG//2 (~35us in)
                nc.scalar.dma_start(mrepB[:], mask_bcast_src(HT, IT))

        # ---- x stream (sync ring), chunk-major in consumption order ----
        xq = [[xtp.tile([P, QI, NFREE], BF16, name=f"xq_{c}_{ng}",
                        tag=f"xq_{c}_{ng}") for ng in range(NG)]
              for c in range(NQ)]
        for c in range(NQ):
            for ng in range(NG):
                nc.sync.dma_start(xq[c][ng][:], xq_d[c, ng])
        # Bias rides the sync ring after x; first needed at ~65us.
        bias_sb = const.tile([P, OT], F32)
        nc.sync.dma_start(bias_sb[:], bias_d[:])

        def xq_slice(it, ng):
            return xq[it // QI][ng][:, it % QI, :]

        def make_wm(ot, tg):
            wm = wtm.tile([P, 4, P], BF16, tag="wtm")
            wsrc = wpiece[(ot, tg)]
            mr, toff = (mrepA, tg * 4) if tg < TG // 2 else \
                (mrepB, tg * 4 - HT)
            m_ap = mr[:, toff:toff + 4, ot * 4:ot * 4 + 4] \
                .broadcast_to([P, 4, 4, BS])
            nc.vector.tensor_tensor(
                wm[:].rearrange("p a (b c) -> p a b c", c=BS),
                wsrc[:].rearrange("p a (b c) -> p a b c", c=BS),
                m_ap, op=mybir.AluOpType.mult)
            return wm

        def mm_group(po, tg, wm, first, last, ngs=None):
            for j in range(4):
                it = tg * 4 + j
                for ng in (range(NG) if ngs is None else ngs):
                    nc.tensor.matmul(po[ng][:], wm[:, j, :], xq_slice(it, ng),
                                     start=(first and j == 0),
                                     stop=(last and j == 3))

        def evict(po, ot, ngs=None, eng="scalar"):
            # eng="vector": bias-add on DVE.  Used for the phase-A
            # evictions so they cannot be scheduled behind ring-gated DMA
            # issue ops on the scalar queue (PSUM banks must free fast for
            # the first phase-B tiles).
            for ng in (range(NG) if ngs is None else ngs):
                ob_t = osb.tile([P, NFREE], F32, tag="osb")
                if eng == "vector":
                    nc.vector.tensor_tensor(
                        ob_t[:], po[ng][:],
                        bias_sb[:, ot:ot + 1].broadcast_to([P, NFREE]),
                        op=mybir.AluOpType.add)
                else:
                    nc.scalar.activation(ob_t[:], po[ng][:],
                                         mybir.ActivationFunctionType.Identity,
                                         bias=bias_sb[:, ot:ot + 1], scale=1.0)
                nc.sync.dma_start(
                    out_d[ot * P:(ot + 1) * P, ng * NFREE:(ng + 1) * NFREE],
                    ob_t[:])

        # ---- phase A: output tiles 0..AOT-1 interleaved across x chunks ----
        poA = {ot: [ppo.tile([P, NFREE], F32, tag="ppo",
                             name=f"poA_{ot}_{ng}") for ng in range(NG)]
               for ot in range(AOT)}
        for c in range(NQ):
            tg = c  # chunk c holds exactly the i-tiles of tile group c
            if c == 0:
                # ng-split: all ng=0 passes first so the first matmuls only
                # need xq[0][0], which lands ~3us before xq[0][1].
                wms = [make_wm(ot, tg) for ot in range(AOT)]
                for ng in range(NG):
                    for ot in range(AOT):
                        mm_group(poA[ot], tg, wms[ot], first=True, last=False,
                                 ngs=[ng])
                continue
            if c == NQ - 1:
                # Last round: build all masked-weight tiles first so the
                # DVE evictions (emitted per-ot below) cannot delay them;
                # banks then free for phase B during this round.
                wms = [make_wm(ot, tg) for ot in range(AOT)]
                for ot in range(AOT):
                    mm_group(poA[ot], tg, wms[ot], first=False, last=True)
                    evict(poA[ot], ot, eng="vector")
                continue
            for ot in range(AOT):
                wm = make_wm(ot, tg)
                mm_group(poA[ot], tg, wm, first=False, last=False)
            # Prefetch the first phase-B weight tiles mid-phase-A so their
            # issue ops (and any buffer waits) clear the scalar queue before
            # the phase-A evictions enter it.
            if c == 3 or c == 5:
                pot = AOT + (c - 3) // 2
                for pc in range(NPC):
                    wpiece[(pot, pc)] = load_piece(pot, pc)

        # ---- phase B: remaining output tiles, x fully resident ----
        # Weight pieces are prefetched two output tiles ahead.
        for ot in range(AOT, OT):
            if ot + 2 < OT:
                for pc in range(NPC):
                    wpiece[(ot + 2, pc)] = load_piece(ot + 2, pc)
            po = [ppo.tile([P, NFREE], F32, tag="ppo", name=f"po_{ot}_{ng}")
                  for ng in range(NG)]
            if ot == OT - 1:
                # ng-split so ng=0's eviction/store overlaps ng=1's matmuls,
                # trimming the kernel tail.
                wms = [make_wm(ot, tg) for tg in range(TG)]
                for ng in range(NG):
                    for tg in range(TG):
                        mm_group(po, tg, wms[tg], first=(tg == 0),
                                 last=(tg == TG - 1), ngs=[ng])
                    evict(po, ot, ngs=[ng])
            else:
                for tg in range(TG):
                    wm = make_wm(ot, tg)
                    mm_group(po, tg, wm, first=(tg == 0), last=(tg == TG - 1))
                evict(po, ot)

    nc.finalize()
    return nc


def _tile_x(x_slice_bf, IN, n_rows):
    """xq[c, ng, p, it, n] = x[ng*NFREE+n, (c*QI+it)*128+p] (bf16 in/out)."""
    QI = 4
    NQ = (IN // P) // QI
    NFREE = min(512, n_rows)
    NG = n_rows // NFREE
    xt = x_slice_bf.T                                  # [IN, n_rows]
    xq = xt.reshape(NQ, QI, P, NG, NFREE).transpose(0, 3, 2, 1, 4)
    return np.ascontiguousarray(xq)


def _install_profile_hook():
    """Provide antenv.axon_hooks + the ctypes NTFF hook (profiling only)."""
    import types

    try:
        from antenv import axon_hooks  # noqa: F401
    except ImportError:
        import antenv

        mod = types.ModuleType("antenv.axon_hooks")
        _h = [None]
        mod.set_axon_ntff_profile_hook = lambda h: _h.__setitem__(0, h)
        mod.get_axon_ntff_profile_hook = lambda: _h[0]
        sys.modules["antenv.axon_hooks"] = mod
        antenv.axon_hooks = mod
    from antenv.axon_hooks import (
        get_axon_ntff_profile_hook,
        set_axon_ntff_profile_hook,
    )

    if get_axon_ntff_profile_hook() is None:
        so_path = "/opt/axon/libaxon_pjrt.so"
        if os.path.exists(so_path):
            from trn_agent_boot.trn_boot import _ntff_profile_via_ctypes

            set_axon_ntff_profile_hook(_ntff_profile_via_ctypes(so_path))

    # Zero-egress container: artifact upload would fail; keep it local.
    import concourse.bass_utils as bu

    bu.upload_artifacts = lambda tmpdir: tmpdir


def kernel(x, weight, bias, block_mask):
    global LAST_EXEC_TIME_NS, LAST_RESULTS
    x = np.ascontiguousarray(np.asarray(x, dtype=np.float32))
    weight = np.ascontiguousarray(np.asarray(weight, dtype=np.float32))
    bias = np.asarray(bias, dtype=np.float32)
    block_mask = np.ascontiguousarray(np.asarray(block_mask, dtype=np.int32))

    N, IN = x.shape
    OUT = weight.shape[0]
    assert N % N_CORES == 0
    n_rows = N // N_CORES
    IT, OT = IN // P, OUT // P

    bf16 = ml_dtypes.bfloat16
    xb = x.astype(bf16)
    wb = weight.astype(bf16)
    # wq[ot, p, it, o] = weight[ot*128+o, it*128+p]
    wq = np.ascontiguousarray(wb.reshape(OT, P, IT, P).transpose(0, 3, 2, 1))
    # maskB[q, t, ob] = block_mask[ob, 4t + q] (i.e. maskT[4t+q, ob])
    maskT = block_mask.T.astype(bf16)
    maskB = np.ascontiguousarray(
        maskT.reshape(IT, 4, OUT // BS).transpose(1, 0, 2))
    bias_r = np.ascontiguousarray(bias.reshape(OT, P).T)

    nc = _build_program(n_rows, IN, OUT)
    in_maps = [{
        "xq": _tile_x(xb[c * n_rows:(c + 1) * n_rows, :], IN, n_rows),
        "wq": wq,
        "maskB": maskB,
        "bias_r": bias_r,
    } for c in range(N_CORES)]

    trace = bool(int(os.environ.get("BASS_KERNEL_TRACE", "0")))
    if trace:
        _install_profile_hook()
    res = run_bass_kernel_spmd(nc, in_maps, list(range(N_CORES)), trace=trace)
    LAST_EXEC_TIME_NS = res.exec_time_ns
    LAST_RESULTS = res

    out = np.empty((N, OUT), dtype=np.float32)
    for c in range(N_CORES):
        out[c * n_rows:(c + 1) * n_rows, :] = res.results[c]["outT"].T
    return out



# revision 2
# speedup vs baseline: 1.0096x; 1.0096x over previous
"""Block-sparse linear via PE 32x32 sub-array tiling (TRN2, 8 cores).

out = x @ (W * expand(block_mask))^T + bias, computed by SKIPPING inactive
32x32 blocks entirely on the tensor engine:

  * Data-parallel over x rows: each core gets 1024 rows, full W/mask/bias.
  * Per output block-row ob (32 outs), its ~64 active input blocks are
    decomposed into runs; consecutive-ib pairs become K=64 stationaries
    (two x layouts: xA natural, xB shifted 32 partitions, so pairs at any
    parity are partition-contiguous); leftovers are K=32 singles.
  * The PE runs as 16 independent 32x32 sub-arrays: 8 concurrent chains
    (4 column-classes x 2 row-halves) accumulate per-(ob,half) partial
    sums in PSUM strips; measured steady state ~434ns per 8-chain round
    (cell-time floor 426ns) -- LDWEIGHTS fully hidden at 8 loads/round.
  * A post-schedule pass deletes redundant InstLdweights (the ng1 matmul
    reuses the loaded stationary; verified on HW) and thins per-matmul
    PE-sem updates (26ns each serialized) down to the waited-on ones,
    preserving final sem totals.
  * Per 4-ob group: halves reduced A+B (+bias) full-width [128,512] on
    scalar+vector engines, PSUM cycles through 8 banks, out streams as
    outT [4096, 1024] f32.
"""
import os
import sys

import ml_dtypes
import numpy as np

sys.path.insert(0, "/opt/trn_rl_repo")

from contextlib import ExitStack

import concourse.bass as bass
import concourse.mybir as mybir
import concourse.tile as tile
from concourse import bacc
from concourse.bass_utils import run_bass_kernel_spmd

N_CORES = 8
BS = 32
P = 128

LAST_EXEC_TIME_NS = None
LAST_RESULTS = None

F32 = mybir.dt.float32
BF16 = mybir.dt.bfloat16


# --------------------------------------------------------------------------
# schedule construction (host-side, mask dependent)
# --------------------------------------------------------------------------

def build_schedule(mask):
    """mask: [128 ob, 128 ib] bool.  Returns per-ob items.

    item = (kind, copy, h, t, ib0) with kind 'pair'|'single':
      pair:   ibs (ib0, ib0+1); copy 'A' (ib0%4 in {0,2}) or 'B' (ib0%4 in
              {1,3}); h = row half (partitions 64h..64h+63);
              A: t = ib0//4, h = (ib0%4)//2;  B: t = ib0//4, h=((ib0%4)-1)//2
      single: copy 'A', row r = ib0%4, t = ib0//4 (partitions 32r..)
    """
    OBS, IBS = mask.shape
    singles_only = bool(os.environ.get("K2_SINGLES_ONLY"))
    items = []
    for ob in range(OBS):
        act = np.nonzero(mask[ob])[0]
        if singles_only:
            items.append([("single", "A", int(k) % 4, int(k) // 4, int(k))
                          for k in act])
            continue
        obit = []
        i = 0
        while i < len(act):
            j = i
            while j + 1 < len(act) and act[j + 1] == act[j] + 1:
                j += 1
            # run act[i..j]
            k = act[i]
            end = act[j]
            while k <= end:
                if k + 1 <= end:
                    kind, ib0 = "pair", k
                    k += 2
                elif k < IBS - 1:
                    kind, ib0 = "pairZ1", k      # partner k+1 zero-weighted
                    k += 1
                else:
                    kind, ib0 = "pairZ0", k - 1  # partner k-1 zero-weighted
                    k += 1
                m = ib0 % 4
                if m in (0, 2):
                    obit.append((kind, "A", m // 2, ib0 // 4, ib0))
                else:
                    obit.append((kind, "B", (m - 1) // 2, ib0 // 4, ib0))
            i = j + 1
        items.append(obit)
    return items


# --------------------------------------------------------------------------
# post-schedule IR passes (validated in microbenchmarks)
# --------------------------------------------------------------------------

def dedup_ldweights(nc):
    """Remove InstLdweights whose covered 32x32 PE cells already hold the
    identical stationary content (same memref/offset/pattern)."""
    n_del = 0
    for f in nc.m.functions:
        for blk in f.blocks:
            insts = list(blk.instructions)
            state = {}
            to_del = []
            for inst in insts:
                if isinstance(inst, mybir.InstLdweights):
                    ap = inst.ins[0]
                    pos = inst.tile_position or (0, 0)
                    size = inst.tile_size or (128, 128)
                    dims = [list(d) for d in ap.ap]
                    pstride, pnum = dims[0]
                    rest = tuple(tuple(d) for d in dims[1:])
                    nrow = (size[0] + 31) // 32
                    ncol = (size[1] + 31) // 32
                    ok = (pnum == size[0] and size[0] % 32 == 0)
                    cells = {}
                    for a in range(nrow):
                        for b in range(ncol):
                            key = (ap.memref, ap.offset + 32 * a * pstride,
                                   rest, size[1], b)
                            cells[(pos[0] // 32 + a, pos[1] // 32 + b)] = key
                    if ok and all(state.get(c) == k for c, k in cells.items()):
                        si = inst.sync_info
                        if si is None or not si.on_wait:
                            to_del.append(inst)
                            n_del += 1
                            continue
                    for c, k in cells.items():
                        state[c] = k
                    continue
                outs = getattr(inst, "outs", None)
                if outs:
                    wrefs = set()
                    for o in outs:
                        mr = getattr(o, "memref", None)
                        if mr is not None:
                            wrefs.add(mr)
                    if wrefs:
                        for c in list(state):
                            if state[c][0] in wrefs:
                                del state[c]
            for inst in to_del:
                blk.instructions.remove(inst)
    return n_del


def thin_pe_sem_updates(nc):
    """Drop per-matmul PE-sem increments no wait needs; each kept update
    absorbs the dropped ones since the previous kept one (sem-add-imm), so
    cumulative values at kept points -- and the final total -- are
    unchanged."""
    n_drop = 0
    for f in nc.m.functions:
        for blk in f.blocks:
            insts = list(blk.instructions)
            incs = {}
            wait_vals = {}
            mode_ok = {}
            for inst in insts:
                si = inst.sync_info
                if not si:
                    continue
                for u in si.on_update:
                    nm = u.ant_name
                    if not nm.startswith("PE_"):
                        continue
                    if u.update_mode != "sem-inc" or not isinstance(
                            inst, (mybir.InstMatmult, mybir.InstLdweights)):
                        mode_ok[nm] = False
                        continue
                    lst = incs.setdefault(nm, [])
                    cum = (lst[-1][2] if lst else 0) + u.update_value
                    lst.append((inst, u, cum))
                for w in si.on_wait:
                    nm = w.ant_name
                    if nm.startswith("PE_"):
                        if w.wait_mode != "sem-ge-imm":
                            mode_ok[nm] = False
                        wait_vals.setdefault(nm, []).append(w)
            for nm, lst in incs.items():
                if mode_ok.get(nm, True) is False:
                    continue
                ws = wait_vals.get(nm, [])
                needed = sorted({w.wait_value for w in ws})
                keep = set()
                wi = 0
                for j, (inst, u, cum) in enumerate(lst):
                    prev = cum - u.update_value
                    hit = False
                    while wi < len(needed) and needed[wi] <= cum:
                        if needed[wi] > prev:
                            hit = True
                        wi += 1
                    if hit:
                        keep.add(j)
                keep.add(len(lst) - 1)
                prev_cum = 0
                for j in sorted(keep):
                    inst, u, cum = lst[j]
                    delta = cum - prev_cum
                    prev_cum = cum
                    if delta != u.update_value:
                        si = inst.sync_info
                        for x in si.on_update:
                            if x is u:
                                x.update_mode = "sem-add-imm"
                                x.update_value = delta
                        inst.sync_info = si
                for j, (inst, u, cum) in enumerate(lst):
                    if j not in keep:
                        si = inst.sync_info
                        si.on_update = [x for x in si.on_update
                                        if x is not u]
                        inst.sync_info = si
                        n_drop += 1
    return n_drop


# --------------------------------------------------------------------------
# program
# --------------------------------------------------------------------------

def _build_program(sched, n_rows, IN, OUT, SCH=64):
    """sched: list per ob of items.  Weight slots are laid out by this
    function identically to _pack_weights (same traversal)."""
    IT = IN // P          # 32 x tiles
    OB = OUT // BS        # 128 output block rows
    NGRP = OB // 4        # 32 groups of 4 obs
    NFREE = 512
    NG = n_rows // NFREE  # 2

    # ---- chain assignment + slot layout (must match _pack_weights) ----
    # chains[g][c][h] = list of items for ob = 4g + c, half h
    chains = [[[[], []] for _ in range(4)] for _ in range(NGRP)]
    for ob in range(OB):
        g, c = ob // 4, ob % 4
        for it in sched[ob]:
            kind, cp, hr, t, ib0 = it
            h = hr if kind.startswith("pair") else hr // 2
            chains[g][c][h].append(it)
    maxit = int(os.environ.get("K2_MAXIT", "0"))
    pad = bool(os.environ.get("K2_ABL_PAD"))
    for g in range(NGRP):
        for c in range(4):
            for h in range(2):
                chains[g][c][h].sort(key=lambda it: it[3])
                if maxit:
                    del chains[g][c][h][maxit:]
                if not chains[g][c][h]:
                    # dummy single (zero weights) keeps the strip written
                    chains[g][c][h].append(("zero", "A", h, 0, -1))
        if pad:
            gmax = max(len(chains[g][c][h])
                       for c in range(4) for h in range(2))
            for c in range(4):
                for h in range(2):
                    while len(chains[g][c][h]) < gmax:
                        chains[g][c][h].append(("zero", "A", h, 0, -1))
    # slot s holds up to one chain-h0 item (partitions 0..63) and one
    # chain-h1 item (partitions 64..127) for the same (g, c, round k)
    slot_of = {}
    nslot = 0
    for g in range(NGRP):
        for c in range(4):
            rounds = max(len(chains[g][c][0]), len(chains[g][c][1]))
            for k in range(rounds):
                for h in range(2):
                    if k < len(chains[g][c][h]):
                        slot_of[(g, c, h, k)] = nslot
                nslot += 1
    NCH = (nslot + SCH - 1) // SCH
    nslot_pad = NCH * SCH

    nc = bacc.Bacc("TRN2", target_bir_lowering=False, debug=False,
                   num_devices=N_CORES)
    xA_d = nc.dram_tensor("xA", [IT, P, n_rows], BF16, kind="ExternalInput")
    xB_d = nc.dram_tensor("xB", [IT, P, n_rows], BF16, kind="ExternalInput")
    wq_d = nc.dram_tensor("wq", [NCH, P, SCH, BS], BF16, kind="ExternalInput")
    bias_d = nc.dram_tensor("bias_g", [P, NGRP], F32, kind="ExternalInput")
    out_d = nc.dram_tensor("outT", [OUT, n_rows], F32, kind="ExternalOutput")

    with tile.TileContext(nc) as tc, ExitStack() as ctx:
        const = ctx.enter_context(tc.tile_pool(name="const", bufs=1))
        xtp = ctx.enter_context(tc.tile_pool(name="xt", bufs=1))
        wpool = ctx.enter_context(tc.tile_pool(name="wp", bufs=4))
        spool = ctx.enter_context(tc.tile_pool(name="sp", bufs=6))
        opool = ctx.enter_context(tc.tile_pool(name="op", bufs=6))
        ppo = ctx.enter_context(tc.tile_pool(name="ppo", bufs=8, space="PSUM"))

        # HAM warm-up
        if not os.environ.get("K2_ABL_NOWARM"):
            dum = const.tile([P, P + NFREE], BF16)
            nc.vector.memset(dum[:], 0.0)
            wup = ppo.tile([P, NFREE], F32, tag="ppo", name="warmup")
            for _ in range(9):
                nc.tensor.matmul(wup[:], dum[:, 0:P], dum[:, P:P + NFREE],
                                 start=True, stop=True)

        # x: both copies, tile-by-tile (consumption is ordered by t)
        xA = xtp.tile([P, IT, n_rows], BF16, name="xA")
        xB = xtp.tile([P, IT, n_rows], BF16, name="xB")
        if os.environ.get("K2_ABL_X1"):
            nc.sync.dma_start(
                xA[:].rearrange("p t n -> t p n"), xA_d[:])
            nc.scalar.dma_start(
                xB[:].rearrange("p t n -> t p n"), xB_d[:])
        else:
            for t in range(IT):
                nc.sync.dma_start(xA[:, t, :], xA_d[t])
                nc.scalar.dma_start(xB[:, t, :], xB_d[t])
        bias_sb = const.tile([P, NGRP], F32)
        nc.sync.dma_start(bias_sb[:], bias_d[:])

        # weight chunks, double buffered
        wch = {}

        if os.environ.get("K2_ABL_WBIG"):
            wbig = xtp.tile([P, NCH * SCH, BS], BF16, name="wbig")
            for ci in range(NCH):
                nc.scalar.dma_start(
                    wbig[:, ci * SCH:(ci + 1) * SCH, :], wq_d[ci])
                wch[ci] = wbig[:, ci * SCH:(ci + 1) * SCH, :]

            def load_chunk(ci):
                return wch[ci]
        else:
            def load_chunk(ci):
                wt = wpool.tile([P, SCH, BS], BF16, tag="wq", name=f"wq{ci}")
                nc.scalar.dma_start(wt[:], wq_d[ci])
                return wt

        for ci in range(min(3, NCH)):
            wch[ci] = load_chunk(ci)

        # zero weights tile for dummy items
        zw = const.tile([P, BS], BF16)
        nc.vector.memset(zw[:], 0.0)

        def wslot_ap(s, h, kind):
            ci, co = s // SCH, s % SCH
            wt = wch[ci]
            if kind == "zero":
                return None
            if kind == "pair":
                return wt[64 * h:64 * h + 64, co, :]
            return wt[:, co, :]  # unused

        def x_ap(it, ng):
            kind, cp, hr, t, ib0 = it
            xt = xA if cp == "A" else xB
            return xt[64 * hr:64 * hr + 64, t, ng * NFREE:(ng + 1) * NFREE]

        nomm = bool(os.environ.get("K2_ABL_NOMM"))
        if nomm:
            z = const.tile([P, NFREE], F32)
            nc.vector.memset(z[:], 0.0)
            for g in range(NGRP):
                for ng in range(2):
                    nc.sync.dma_start(
                        out_d[g * P:(g + 1) * P,
                              ng * NFREE:(ng + 1) * NFREE], z[:])

        next_chunk = min(3, NCH)
        barrier_every = int(os.environ.get("K2_BARRIER", "0"))
        # 8 persistent PSUM tiles: [parity][half][ng]; groups alternate
        # parity so eviction of g-1 overlaps accumulation of g.
        pbank = [[[ppo.tile([P, NFREE], F32, tag="ppo",
                            name=f"pb{pa}_{h}_{ng}") for ng in range(2)]
                  for h in range(2)] for pa in range(2)]
        for g in range(NGRP if not nomm else 0):
            if barrier_every and g and g % barrier_every == 0:
                tc.no_sync_barrier()
            bank = pbank[0 if os.environ.get('K2_ABL_4BANK') else g % 2]
            rounds = max(max(len(chains[g][c][0]), len(chains[g][c][1]))
                         for c in range(4))
            started = {}
            nleft = {(c, h): len(chains[g][c][h])
                     for c in range(4) for h in range(2)}
            for k in range(rounds):
                for c in range(4):
                    for h in range(2):
                        cl = chains[g][c][h]
                        if k >= len(cl):
                            continue
                        it = cl[k]
                        kind = it[0]
                        s = slot_of[(g, c, h, k)]
                        # prefetch next chunk when entering last loaded one
                        ci = s // SCH
                        while next_chunk <= ci + 2 and next_chunk < NCH:
                            wch[next_chunk] = load_chunk(next_chunk)
                            next_chunk += 1
                        if kind == "zero":
                            lhsT = zw[64 * h:64 * h + 64, :]
                        else:
                            lhsT = wslot_ap(s, h, "pair")
                        tp = (64 * h, 32 * c)
                        first = (c, h) not in started
                        started[(c, h)] = True
                        last = (k == len(cl) - 1)
                        for ng in range(2):
                            nc.tensor.matmul(
                                bank[h][ng][32 * c:32 * c + 32, :],
                                lhsT, x_ap(it, ng),
                                start=first, stop=last,
                                tile_position=tp, skip_group_check=True)
            # reduce + bias + evict
            if os.environ.get("K2_ABL_NOEV"):
                if g == 0:
                    z2 = const.tile([P, NFREE], F32, name="z2")
                    nc.vector.memset(z2[:], 0.0)
                    for gg in range(NGRP):
                        for ng in range(2):
                            nc.sync.dma_start(
                                out_d[gg * P:(gg + 1) * P,
                                      ng * NFREE:(ng + 1) * NFREE], z2[:])
                continue
            for ng in range(2):
                if os.environ.get("K2_ABL_EVSIMPLE"):
                    o0 = opool.tile([P, NFREE], F32, tag="op")
                    nc.scalar.activation(
                        o0[:], bank[0][ng][:],
                        mybir.ActivationFunctionType.Identity, scale=1.0)
                    nc.vector.tensor_tensor(o0[:], o0[:], bank[1][ng][:],
                                            op=mybir.AluOpType.add)
                    nc.sync.dma_start(
                        out_d[g * P:(g + 1) * P,
                              ng * NFREE:(ng + 1) * NFREE], o0[:])
                    continue
                t0 = spool.tile([P, NFREE], F32, tag="sp")
                nc.scalar.activation(t0[:], bank[0][ng][:],
                                     mybir.ActivationFunctionType.Identity,
                                     bias=bias_sb[:, g:g + 1], scale=1.0)
                o0 = opool.tile([P, NFREE], F32, tag="op")
                nc.vector.tensor_tensor(o0[:], t0[:], bank[1][ng][:],
                                        op=mybir.AluOpType.add)
                nc.sync.dma_start(
                    out_d[g * P:(g + 1) * P, ng * NFREE:(ng + 1) * NFREE],
                    o0[:])

    n_del = 0 if os.environ.get("K2_NO_DEDUP") else dedup_ldweights(nc)
    n_thin = 0 if os.environ.get("K2_NO_THIN") else thin_pe_sem_updates(nc)
    if os.environ.get("K2_DEBUG"):
        print(f"kernel2: {nslot} slots, dedup {n_del} LDW, thin {n_thin}")
    nc.finalize()
    return nc, nslot_pad


def _pack_weights(sched, weight, IN, OUT, SCH=64):
    """Must mirror _build_program's traversal exactly."""
    OB = OUT // BS
    NGRP = OB // 4
    chains = [[[[], []] for _ in range(4)] for _ in range(NGRP)]
    for ob in range(OB):
        g, c = ob // 4, ob % 4
        for it in sched[ob]:
            kind, cp, hr, t, ib0 = it
            h = hr if kind.startswith("pair") else hr // 2
            chains[g][c][h].append(it)
    slots = []
    maxit = int(os.environ.get("K2_MAXIT", "0"))
    pad = bool(os.environ.get("K2_ABL_PAD"))
    for g in range(NGRP):
        for c in range(4):
            for h in range(2):
                chains[g][c][h].sort(key=lambda it: it[3])
                if maxit:
                    del chains[g][c][h][maxit:]
                if not chains[g][c][h]:
                    chains[g][c][h].append(("zero", "A", h, 0, -1))
        if pad:
            gmax = max(len(chains[g][c][h])
                       for c in range(4) for h in range(2))
            for c in range(4):
                for h in range(2):
                    while len(chains[g][c][h]) < gmax:
                        chains[g][c][h].append(("zero", "A", h, 0, -1))
    for g in range(NGRP):
        for c in range(4):
            rounds = max(len(chains[g][c][0]), len(chains[g][c][1]))
            for k in range(rounds):
                ent = []
                for h in range(2):
                    if k < len(chains[g][c][h]):
                        ent.append((4 * g + c, h, chains[g][c][h][k]))
                slots.append(ent)
    nslot = len(slots)
    NCH = (nslot + SCH - 1) // SCH
    bf16 = ml_dtypes.bfloat16
    wq = np.zeros((NCH * SCH, P, BS), np.float32)
    for s, ent in enumerate(slots):
        for ob, h, it in ent:
            kind, cp, hr, t, ib0 = it
            if kind == "zero":
                continue
            # stationary [64, 32]: rows 0..31 = ib0, 32..63 = ib0+1
            if kind != "pairZ0":
                blk0 = weight[ob * BS:(ob + 1) * BS,
                              ib0 * BS:(ib0 + 1) * BS]      # [o, i]
                wq[s, 64 * h:64 * h + 32, :] = blk0.T
            if kind != "pairZ1":
                blk1 = weight[ob * BS:(ob + 1) * BS,
                              (ib0 + 1) * BS:(ib0 + 2) * BS]
                wq[s, 64 * h + 32:64 * h + 64, :] = blk1.T
    return np.ascontiguousarray(wq.reshape(NCH, SCH, P, BS)
                                .transpose(0, 2, 1, 3).astype(bf16))


def _install_profile_hook():
    import types

    try:
        from antenv import axon_hooks  # noqa: F401
    except ImportError:
        import antenv

        mod = types.ModuleType("antenv.axon_hooks")
        _h = [None]
        mod.set_axon_ntff_profile_hook = lambda h: _h.__setitem__(0, h)
        mod.get_axon_ntff_profile_hook = lambda: _h[0]
        sys.modules["antenv.axon_hooks"] = mod
        antenv.axon_hooks = mod
    from antenv.axon_hooks import (
        get_axon_ntff_profile_hook,
        set_axon_ntff_profile_hook,
    )

    if get_axon_ntff_profile_hook() is None:
        so_path = "/opt/axon/libaxon_pjrt.so"
        if os.path.exists(so_path):
            from trn_agent_boot.trn_boot import _ntff_profile_via_ctypes

            set_axon_ntff_profile_hook(_ntff_profile_via_ctypes(so_path))

    import concourse.bass_utils as bu

    bu.upload_artifacts = lambda tmpdir: tmpdir


def kernel(x, weight, bias, block_mask):
    global LAST_EXEC_TIME_NS, LAST_RESULTS
    x = np.ascontiguousarray(np.asarray(x, dtype=np.float32))
    weight = np.ascontiguousarray(np.asarray(weight, dtype=np.float32))
    bias = np.asarray(bias, dtype=np.float32)
    block_mask = np.ascontiguousarray(np.asarray(block_mask, dtype=np.int32))

    N, IN = x.shape
    OUT = weight.shape[0]
    n_rows = N // N_CORES
    IT = IN // P

    mask = block_mask > 0
    sched = build_schedule(mask)

    bf16 = ml_dtypes.bfloat16
    xb = x.astype(bf16)
    wq = _pack_weights(sched, weight, IN, OUT)
    # bias_g[p, g] = bias[g*128 + p] (group g covers obs 4g..4g+3 = outs
    # 128g..128g+127, strip c at partitions 32c)
    bias_g = np.ascontiguousarray(bias.reshape(OUT // P, P).T)

    nc, nslot_pad = _build_program(sched, n_rows, IN, OUT)

    in_maps = []
    for cix in range(N_CORES):
        xs = xb[cix * n_rows:(cix + 1) * n_rows, :]       # [n_rows, IN]
        xT = np.ascontiguousarray(xs.T)                   # [IN, n_rows]
        xA = np.ascontiguousarray(xT.reshape(IT, P, n_rows))
        xT2 = np.zeros_like(xT)
        xT2[:IN - BS] = xT[BS:]
        xB = np.ascontiguousarray(xT2.reshape(IT, P, n_rows))
        in_maps.append({"xA": xA, "xB": xB, "wq": wq, "bias_g": bias_g})

    trace = bool(int(os.environ.get("BASS_KERNEL_TRACE", "0")))
    if trace:
        _install_profile_hook()
    res = run_bass_kernel_spmd(nc, in_maps, list(range(N_CORES)), trace=trace)
    LAST_EXEC_TIME_NS = res.exec_time_ns
    LAST_RESULTS = res

    out = np.empty((N, OUT), dtype=np.float32)
    for cix in range(N_CORES):
        out[cix * n_rows:(cix + 1) * n_rows, :] = res.results[cix]["outT"].T
    return out


# revision 6
# speedup vs baseline: 1.0145x; 1.0049x over previous
"""Block-sparse linear via PE 32x32 sub-array tiling (TRN2, 8 cores).

out = x @ (W * expand(block_mask))^T + bias, computed by SKIPPING inactive
32x32 blocks entirely on the tensor engine:

  * Data-parallel over x rows: each core gets 1024 rows, full W/mask/bias.
  * Per output block-row ob (32 outs), its ~64 active input blocks are
    decomposed into runs; consecutive-ib pairs become K=64 stationaries
    (two x layouts: xA natural, xB shifted 32 partitions, so pairs at any
    parity are partition-contiguous); leftovers are K=32 singles.
  * The PE runs as 16 independent 32x32 sub-arrays: 8 concurrent chains
    (4 column-classes x 2 row-halves) accumulate per-(ob,half) partial
    sums in PSUM strips; measured steady state ~434ns per 8-chain round
    (cell-time floor 426ns) -- LDWEIGHTS fully hidden at 8 loads/round.
  * A post-schedule pass deletes redundant InstLdweights (the ng1 matmul
    reuses the loaded stationary; verified on HW) and thins per-matmul
    PE-sem updates (26ns each serialized) down to the waited-on ones,
    preserving final sem totals.
  * Per 4-ob group: halves reduced A+B (+bias) full-width [128,512] on
    scalar+vector engines, PSUM cycles through 8 banks, out streams as
    outT [4096, 1024] f32.
"""
import os
import sys

import ml_dtypes
import numpy as np

sys.path.insert(0, "/opt/trn_rl_repo")

from contextlib import ExitStack

import concourse.bass as bass
import concourse.mybir as mybir
import concourse.tile as tile
from concourse import bacc
from concourse.bass_utils import run_bass_kernel_spmd

N_CORES = 8
BS = 32
P = 128

LAST_EXEC_TIME_NS = None
LAST_RESULTS = None

F32 = mybir.dt.float32
BF16 = mybir.dt.bfloat16


# --------------------------------------------------------------------------
# schedule construction (host-side, mask dependent)
# --------------------------------------------------------------------------

def build_schedule(mask):
    """mask: [128 ob, 128 ib] bool.  Returns per-ob items.

    item = (kind, copy, h, t, ib0) with kind 'pair'|'single':
      pair:   ibs (ib0, ib0+1); copy 'A' (ib0%4 in {0,2}) or 'B' (ib0%4 in
              {1,3}); h = row half (partitions 64h..64h+63);
              A: t = ib0//4, h = (ib0%4)//2;  B: t = ib0//4, h=((ib0%4)-1)//2
      single: copy 'A', row r = ib0%4, t = ib0//4 (partitions 32r..)
    """
    OBS, IBS = mask.shape
    singles_only = bool(os.environ.get("K2_SINGLES_ONLY"))
    items = []
    for ob in range(OBS):
        act = np.nonzero(mask[ob])[0]
        if singles_only:
            items.append([("single", "A", int(k) % 4, int(k) // 4, int(k))
                          for k in act])
            continue
        obit = []
        i = 0
        while i < len(act):
            j = i
            while j + 1 < len(act) and act[j + 1] == act[j] + 1:
                j += 1
            # run act[i..j]
            k = act[i]
            end = act[j]
            while k <= end:
                if k + 1 <= end:
                    kind, ib0 = "pair", k
                    k += 2
                elif k < IBS - 1:
                    kind, ib0 = "pairZ1", k      # partner k+1 zero-weighted
                    k += 1
                else:
                    kind, ib0 = "pairZ0", k - 1  # partner k-1 zero-weighted
                    k += 1
                m = ib0 % 4
                if m in (0, 2):
                    obit.append((kind, "A", m // 2, ib0 // 4, ib0))
                else:
                    obit.append((kind, "B", (m - 1) // 2, ib0 // 4, ib0))
            i = j + 1
        items.append(obit)
    return items


# --------------------------------------------------------------------------
# post-schedule IR passes (validated in microbenchmarks)
# --------------------------------------------------------------------------

def dedup_ldweights(nc):
    """Remove InstLdweights whose covered 32x32 PE cells already hold the
    identical stationary content (same memref/offset/pattern)."""
    n_del = 0
    for f in nc.m.functions:
        for blk in f.blocks:
            insts = list(blk.instructions)
            state = {}
            to_del = []
            for inst in insts:
                if isinstance(inst, mybir.InstLdweights):
                    ap = inst.ins[0]
                    pos = inst.tile_position or (0, 0)
                    size = inst.tile_size or (128, 128)
                    dims = [list(d) for d in ap.ap]
                    pstride, pnum = dims[0]
                    rest = tuple(tuple(d) for d in dims[1:])
                    nrow = (size[0] + 31) // 32
                    ncol = (size[1] + 31) // 32
                    ok = (pnum == size[0] and size[0] % 32 == 0)
                    cells = {}
                    for a in range(nrow):
                        for b in range(ncol):
                            key = (ap.memref, ap.offset + 32 * a * pstride,
                                   rest, size[1], b)
                            cells[(pos[0] // 32 + a, pos[1] // 32 + b)] = key
                    if ok and all(state.get(c) == k for c, k in cells.items()):
                        si = inst.sync_info
                        if si is None or not si.on_wait:
                            to_del.append(inst)
                            n_del += 1
                            continue
                    for c, k in cells.items():
                        state[c] = k
                    continue
                outs = getattr(inst, "outs", None)
                if outs:
                    wrefs = set()
                    for o in outs:
                        mr = getattr(o, "memref", None)
                        if mr is not None:
                            wrefs.add(mr)
                    if wrefs:
                        for c in list(state):
                            if state[c][0] in wrefs:
                                del state[c]
            for inst in to_del:
                blk.instructions.remove(inst)
    return n_del


def thin_pe_sem_updates(nc):
    """Drop per-matmul PE-sem increments no wait needs; each kept update
    absorbs the dropped ones since the previous kept one (sem-add-imm), so
    cumulative values at kept points -- and the final total -- are
    unchanged."""
    n_drop = 0
    for f in nc.m.functions:
        for blk in f.blocks:
            insts = list(blk.instructions)
            incs = {}
            wait_vals = {}
            mode_ok = {}
            for inst in insts:
                si = inst.sync_info
                if not si:
                    continue
                for u in si.on_update:
                    nm = u.ant_name
                    if not nm.startswith("PE_"):
                        continue
                    if u.update_mode != "sem-inc" or not isinstance(
                            inst, (mybir.InstMatmult, mybir.InstLdweights)):
                        mode_ok[nm] = False
                        continue
                    lst = incs.setdefault(nm, [])
                    cum = (lst[-1][2] if lst else 0) + u.update_value
                    lst.append((inst, u, cum))
                for w in si.on_wait:
                    nm = w.ant_name
                    if nm.startswith("PE_"):
                        if w.wait_mode != "sem-ge-imm":
                            mode_ok[nm] = False
                        wait_vals.setdefault(nm, []).append(w)
            for nm, lst in incs.items():
                if mode_ok.get(nm, True) is False:
                    continue
                ws = wait_vals.get(nm, [])
                needed = sorted({w.wait_value for w in ws})
                keep = set()
                wi = 0
                for j, (inst, u, cum) in enumerate(lst):
                    prev = cum - u.update_value
                    hit = False
                    while wi < len(needed) and needed[wi] <= cum:
                        if needed[wi] > prev:
                            hit = True
                        wi += 1
                    if hit:
                        keep.add(j)
                keep.add(len(lst) - 1)
                prev_cum = 0
                for j in sorted(keep):
                    inst, u, cum = lst[j]
                    delta = cum - prev_cum
                    prev_cum = cum
                    if delta != u.update_value:
                        si = inst.sync_info
                        for x in si.on_update:
                            if x is u:
                                x.update_mode = "sem-add-imm"
                                x.update_value = delta
                        inst.sync_info = si
                for j, (inst, u, cum) in enumerate(lst):
                    if j not in keep:
                        si = inst.sync_info
                        si.on_update = [x for x in si.on_update
                                        if x is not u]
                        inst.sync_info = si
                        n_drop += 1
    return n_drop


# --------------------------------------------------------------------------
# program
# --------------------------------------------------------------------------

def _build_program(sched, n_rows, IN, OUT, SCH=64):
    """sched: list per ob of items.  Weight slots are laid out by this
    function identically to _pack_weights (same traversal)."""
    IT = IN // P          # 32 x tiles
    OB = OUT // BS        # 128 output block rows
    NGRP = OB // 4        # 32 groups of 4 obs
    NFREE = 512
    NG = n_rows // NFREE  # 2

    # ---- chain assignment + slot layout (must match _pack_weights) ----
    # chains[g][c][h] = list of items for ob = 4g + c, half h
    chains = [[[[], []] for _ in range(4)] for _ in range(NGRP)]
    for ob in range(OB):
        g, c = ob // 4, ob % 4
        for it in sched[ob]:
            kind, cp, hr, t, ib0 = it
            h = hr if kind.startswith("pair") else hr // 2
            chains[g][c][h].append(it)
    maxit = int(os.environ.get("K2_MAXIT", "0"))
    pad = bool(os.environ.get("K2_ABL_PAD"))
    for g in range(NGRP):
        for c in range(4):
            for h in range(2):
                chains[g][c][h].sort(key=lambda it: it[3])
                if maxit:
                    del chains[g][c][h][maxit:]
                if not chains[g][c][h]:
                    # dummy single (zero weights) keeps the strip written
                    chains[g][c][h].append(("zero", "A", h, 0, -1))
        if pad:
            gmax = max(len(chains[g][c][h])
                       for c in range(4) for h in range(2))
            for c in range(4):
                for h in range(2):
                    while len(chains[g][c][h]) < gmax:
                        chains[g][c][h].append(("zero", "A", h, 0, -1))
    # slot s holds up to one chain-h0 item (partitions 0..63) and one
    # chain-h1 item (partitions 64..127) for the same (g, c, round k)
    slot_of = {}
    nslot = 0
    for g in range(NGRP):
        for c in range(4):
            rounds = max(len(chains[g][c][0]), len(chains[g][c][1]))
            for k in range(rounds):
                for h in range(2):
                    if k < len(chains[g][c][h]):
                        slot_of[(g, c, h, k)] = nslot
                nslot += 1
    NCH = (nslot + SCH - 1) // SCH
    nslot_pad = NCH * SCH

    nc = bacc.Bacc("TRN2", target_bir_lowering=False, debug=False,
                   num_devices=N_CORES)
    xA_d = nc.dram_tensor("xA", [IT, P, n_rows], BF16, kind="ExternalInput")
    xB_d = nc.dram_tensor("xB", [IT, P, n_rows], BF16, kind="ExternalInput")
    wq_d = nc.dram_tensor("wq", [NCH, P, SCH, BS], BF16, kind="ExternalInput")
    bias_d = nc.dram_tensor("bias_g", [P, NGRP], F32, kind="ExternalInput")
    out_d = nc.dram_tensor("outT", [OUT, n_rows], F32, kind="ExternalOutput")

    with tile.TileContext(nc) as tc, ExitStack() as ctx:
        const = ctx.enter_context(tc.tile_pool(name="const", bufs=1))
        xtp = ctx.enter_context(tc.tile_pool(name="xt", bufs=1))
        wpool = ctx.enter_context(tc.tile_pool(name="wp", bufs=4))
        spool = ctx.enter_context(tc.tile_pool(name="sp", bufs=6))
        opool = ctx.enter_context(tc.tile_pool(name="op", bufs=6))
        ppo = ctx.enter_context(tc.tile_pool(name="ppo", bufs=8, space="PSUM"))

        # HAM warm-up
        if not os.environ.get("K2_ABL_NOWARM"):
            dum = const.tile([P, P + NFREE], BF16)
            nc.vector.memset(dum[:], 0.0)
            wup = ppo.tile([P, NFREE], F32, tag="ppo", name="warmup")
            for _ in range(9):
                nc.tensor.matmul(wup[:], dum[:, 0:P], dum[:, P:P + NFREE],
                                 start=True, stop=True)

        # x: both copies, tile-by-tile (consumption is ordered by t)
        xA = xtp.tile([P, IT, n_rows], BF16, name="xA")
        xB = xtp.tile([P, IT, n_rows], BF16, name="xB")
        bias_sb = const.tile([P, NGRP], F32)
        nc.sync.dma_start(bias_sb[:], bias_d[:])
        for t in range(IT):
            nc.sync.dma_start(xA[:, t, :], xA_d[t])
            nc.scalar.dma_start(xB[:, t, :], xB_d[t])

        # weight chunks, double buffered
        wch = {}

        if os.environ.get("K2_ABL_WBIG"):
            wbig = xtp.tile([P, NCH * SCH, BS], BF16, name="wbig")
            for ci in range(NCH):
                nc.scalar.dma_start(
                    wbig[:, ci * SCH:(ci + 1) * SCH, :], wq_d[ci])
                wch[ci] = wbig[:, ci * SCH:(ci + 1) * SCH, :]

            def load_chunk(ci):
                return wch[ci]
        else:
            def load_chunk(ci):
                wt = wpool.tile([P, SCH, BS], BF16, tag="wq", name=f"wq{ci}")
                nc.gpsimd.dma_start(wt[:], wq_d[ci])
                return wt

        for ci in range(min(3, NCH)):
            wch[ci] = load_chunk(ci)

        # zero weights tile for dummy items
        zw = const.tile([P, BS], BF16)
        nc.vector.memset(zw[:], 0.0)

        def wslot_ap(s, h, kind):
            ci, co = s // SCH, s % SCH
            wt = wch[ci]
            if kind == "zero":
                return None
            if kind == "pair":
                return wt[64 * h:64 * h + 64, co, :]
            return wt[:, co, :]  # unused

        def x_ap(it, ng):
            kind, cp, hr, t, ib0 = it
            xt = xA if cp == "A" else xB
            return xt[64 * hr:64 * hr + 64, t, ng * NFREE:(ng + 1) * NFREE]

        nomm = bool(os.environ.get("K2_ABL_NOMM"))
        if nomm:
            z = const.tile([P, NFREE], F32)
            nc.vector.memset(z[:], 0.0)
            for g in range(NGRP):
                for ng in range(2):
                    nc.sync.dma_start(
                        out_d[g * P:(g + 1) * P,
                              ng * NFREE:(ng + 1) * NFREE], z[:])

        next_chunk = min(3, NCH)
        barrier_every = int(os.environ.get("K2_BARRIER", "0"))
        # 8 persistent PSUM tiles: [parity][half][ng]; groups alternate
        # parity so eviction of g-1 overlaps accumulation of g.
        pbank = [[[ppo.tile([P, NFREE], F32, tag="ppo",
                            name=f"pb{pa}_{h}_{ng}") for ng in range(2)]
                  for h in range(2)] for pa in range(2)]
        for g in range(NGRP if not nomm else 0):
            if barrier_every and g and g % barrier_every == 0:
                tc.no_sync_barrier()
            bank = pbank[0 if os.environ.get('K2_ABL_4BANK') else g % 2]
            rounds = max(max(len(chains[g][c][0]), len(chains[g][c][1]))
                         for c in range(4))
            started = {}
            nleft = {(c, h): len(chains[g][c][h])
                     for c in range(4) for h in range(2)}
            for k in range(rounds):
                for c in range(4):
                    for h in range(2):
                        cl = chains[g][c][h]
                        if k >= len(cl):
                            continue
                        it = cl[k]
                        kind = it[0]
                        s = slot_of[(g, c, h, k)]
                        # prefetch next chunk when entering last loaded one
                        ci = s // SCH
                        while next_chunk <= ci + 2 and next_chunk < NCH:
                            wch[next_chunk] = load_chunk(next_chunk)
                            next_chunk += 1
                        if kind == "zero":
                            lhsT = zw[64 * h:64 * h + 64, :]
                        else:
                            lhsT = wslot_ap(s, h, "pair")
                        tp = (64 * h, 32 * c)
                        first = (c, h) not in started
                        started[(c, h)] = True
                        last = (k == len(cl) - 1)
                        for ng in range(2):
                            nc.tensor.matmul(
                                bank[h][ng][32 * c:32 * c + 32, :],
                                lhsT, x_ap(it, ng),
                                start=first, stop=last,
                                tile_position=tp, skip_group_check=True)
            # reduce + bias + evict
            if os.environ.get("K2_ABL_NOEV"):
                if g == 0:
                    z2 = const.tile([P, NFREE], F32, name="z2")
                    nc.vector.memset(z2[:], 0.0)
                    for gg in range(NGRP):
                        for ng in range(2):
                            nc.sync.dma_start(
                                out_d[gg * P:(gg + 1) * P,
                                      ng * NFREE:(ng + 1) * NFREE], z2[:])
                continue
            for ng in range(2):
                if os.environ.get("K2_ABL_EVSIMPLE"):
                    o0 = opool.tile([P, NFREE], F32, tag="op")
                    nc.scalar.activation(
                        o0[:], bank[0][ng][:],
                        mybir.ActivationFunctionType.Identity, scale=1.0)
                    nc.vector.tensor_tensor(o0[:], o0[:], bank[1][ng][:],
                                            op=mybir.AluOpType.add)
                    nc.sync.dma_start(
                        out_d[g * P:(g + 1) * P,
                              ng * NFREE:(ng + 1) * NFREE], o0[:])
                    continue
                t0 = spool.tile([P, NFREE], F32, tag="sp")
                nc.scalar.activation(t0[:], bank[0][ng][:],
                                     mybir.ActivationFunctionType.Identity,
                                     bias=bias_sb[:, g:g + 1], scale=1.0)
                o0 = opool.tile([P, NFREE], F32, tag="op")
                nc.vector.tensor_tensor(o0[:], t0[:], bank[1][ng][:],
                                        op=mybir.AluOpType.add)
                nc.sync.dma_start(
                    out_d[g * P:(g + 1) * P, ng * NFREE:(ng + 1) * NFREE],
                    o0[:])

    n_del = 0 if os.environ.get("K2_NO_DEDUP") else dedup_ldweights(nc)
    n_thin = 0 if os.environ.get("K2_NO_THIN") else thin_pe_sem_updates(nc)
    if os.environ.get("K2_DEBUG"):
        print(f"kernel2: {nslot} slots, dedup {n_del} LDW, thin {n_thin}")
    nc.finalize()
    return nc, nslot_pad


def _pack_weights(sched, weight, IN, OUT, SCH=64):
    """Must mirror _build_program's traversal exactly."""
    OB = OUT // BS
    NGRP = OB // 4
    chains = [[[[], []] for _ in range(4)] for _ in range(NGRP)]
    for ob in range(OB):
        g, c = ob // 4, ob % 4
        for it in sched[ob]:
            kind, cp, hr, t, ib0 = it
            h = hr if kind.startswith("pair") else hr // 2
            chains[g][c][h].append(it)
    slots = []
    maxit = int(os.environ.get("K2_MAXIT", "0"))
    pad = bool(os.environ.get("K2_ABL_PAD"))
    for g in range(NGRP):
        for c in range(4):
            for h in range(2):
                chains[g][c][h].sort(key=lambda it: it[3])
                if maxit:
                    del chains[g][c][h][maxit:]
                if not chains[g][c][h]:
                    chains[g][c][h].append(("zero", "A", h, 0, -1))
        if pad:
            gmax = max(len(chains[g][c][h])
                       for c in range(4) for h in range(2))
            for c in range(4):
                for h in range(2):
                    while len(chains[g][c][h]) < gmax:
                        chains[g][c][h].append(("zero", "A", h, 0, -1))
    for g in range(NGRP):
        for c in range(4):
            rounds = max(len(chains[g][c][0]), len(chains[g][c][1]))
            for k in range(rounds):
                ent = []
                for h in range(2):
                    if k < len(chains[g][c][h]):
                        ent.append((4 * g + c, h, chains[g][c][h][k]))
                slots.append(ent)
    nslot = len(slots)
    NCH = (nslot + SCH - 1) // SCH
    bf16 = ml_dtypes.bfloat16
    wq = np.zeros((NCH * SCH, P, BS), np.float32)
    for s, ent in enumerate(slots):
        for ob, h, it in ent:
            kind, cp, hr, t, ib0 = it
            if kind == "zero":
                continue
            # stationary [64, 32]: rows 0..31 = ib0, 32..63 = ib0+1
            if kind != "pairZ0":
                blk0 = weight[ob * BS:(ob + 1) * BS,
                              ib0 * BS:(ib0 + 1) * BS]      # [o, i]
                wq[s, 64 * h:64 * h + 32, :] = blk0.T
            if kind != "pairZ1":
                blk1 = weight[ob * BS:(ob + 1) * BS,
                              (ib0 + 1) * BS:(ib0 + 2) * BS]
                wq[s, 64 * h + 32:64 * h + 64, :] = blk1.T
    return np.ascontiguousarray(wq.reshape(NCH, SCH, P, BS)
                                .transpose(0, 2, 1, 3).astype(bf16))


def _install_profile_hook():
    import types

    try:
        from antenv import axon_hooks  # noqa: F401
    except ImportError:
        import antenv

        mod = types.ModuleType("antenv.axon_hooks")
        _h = [None]
        mod.set_axon_ntff_profile_hook = lambda h: _h.__setitem__(0, h)
        mod.get_axon_ntff_profile_hook = lambda: _h[0]
        sys.modules["antenv.axon_hooks"] = mod
        antenv.axon_hooks = mod
    from antenv.axon_hooks import (
        get_axon_ntff_profile_hook,
        set_axon_ntff_profile_hook,
    )

    if get_axon_ntff_profile_hook() is None:
        so_path = "/opt/axon/libaxon_pjrt.so"
        if os.path.exists(so_path):
            from trn_agent_boot.trn_boot import _ntff_profile_via_ctypes

            set_axon_ntff_profile_hook(_ntff_profile_via_ctypes(so_path))

    import concourse.bass_utils as bu

    bu.upload_artifacts = lambda tmpdir: tmpdir


def kernel(x, weight, bias, block_mask):
    global LAST_EXEC_TIME_NS, LAST_RESULTS
    x = np.ascontiguousarray(np.asarray(x, dtype=np.float32))
    weight = np.ascontiguousarray(np.asarray(weight, dtype=np.float32))
    bias = np.asarray(bias, dtype=np.float32)
    block_mask = np.ascontiguousarray(np.asarray(block_mask, dtype=np.int32))

    N, IN = x.shape
    OUT = weight.shape[0]
    n_rows = N // N_CORES
    IT = IN // P

    mask = block_mask > 0
    sched = build_schedule(mask)

    bf16 = ml_dtypes.bfloat16
    xb = x.astype(bf16)
    wq = _pack_weights(sched, weight, IN, OUT)
    # bias_g[p, g] = bias[g*128 + p] (group g covers obs 4g..4g+3 = outs
    # 128g..128g+127, strip c at partitions 32c)
    bias_g = np.ascontiguousarray(bias.reshape(OUT // P, P).T)

    nc, nslot_pad = _build_program(sched, n_rows, IN, OUT)

    in_maps = []
    for cix in range(N_CORES):
        xs = xb[cix * n_rows:(cix + 1) * n_rows, :]       # [n_rows, IN]
        xT = np.ascontiguousarray(xs.T)                   # [IN, n_rows]
        xA = np.ascontiguousarray(xT.reshape(IT, P, n_rows))
        xT2 = np.zeros_like(xT)
        xT2[:IN - BS] = xT[BS:]
        xB = np.ascontiguousarray(xT2.reshape(IT, P, n_rows))
        in_maps.append({"xA": xA, "xB": xB, "wq": wq, "bias_g": bias_g})

    trace = bool(int(os.environ.get("BASS_KERNEL_TRACE", "0")))
    if trace:
        _install_profile_hook()
    res = run_bass_kernel_spmd(nc, in_maps, list(range(N_CORES)), trace=trace)
    LAST_EXEC_TIME_NS = res.exec_time_ns
    LAST_RESULTS = res

    out = np.empty((N, OUT), dtype=np.float32)
    for cix in range(N_CORES):
        out[cix * n_rows:(cix + 1) * n_rows, :] = res.results[cix]["outT"].T
    return out
